# revision 7
# baseline (speedup 1.0000x reference)
"""Trainium2 Bass kernel for the 4-way additive/bilinear/product/difference
attention module (B=64, T=256, H=768), data-parallel over batch across 8
NeuronCores.

Math per batch b (reference semantics):
  sc[i,j] = tanh((p@Wc2)[i,j] + (q@Wc1)[j,i]) * vc[i];  qc = softmax_j(sc) @ q
  sb[i,j] = (p@Wb@q^T)[i,j];                            qb = softmax_j(sb) @ q
  sd[i,j] = tanh(sum_h p[i,h]Wd[h]q[j,h]) * vd[j];      qd = softmax_j(sd) @ q
  sm[i,j] = tanh((q@Wm)[j] - (p@Wm)[i]) * vm[j];        qm = softmax_j(sm) @ q

On-chip, all score matrices are built TRANSPOSED (S^T[j,i], softmax axis j on
partitions) so the attention matrix lands directly in the lhsT layout needed
for the A@q matmul — no per-batch transposes of A. Softmax denominators are
computed with ones-vector matmuls; only sb needs max-subtraction (a single
global max is numerically valid and safe: measured gmax-rowmax spread < 70).
"""

import os

import numpy as np

B, T, H = 64, 256, 768
NCORES = 8
BPC = B // NCORES  # batches per core
HK = H // 128  # 6 h-chunks
TC = T // 128  # 2 t-chunks
NH = 384  # output free-dim half (PSUM bank limit: 512 f32)

_CACHE = {}

# set by kernel() when BASS_KERNEL_TRACE=1 (read by test harness)
last_exec_time_ns = None
last_trace_dir = None


def _build_program():
    from contextlib import ExitStack

    import concourse.bass as bass
    import concourse.tile as tile
    from concourse import bacc, mybir
    from concourse.masks import make_identity

    f32 = mybir.dt.float32
    AF = mybir.ActivationFunctionType

    # Bacc (not raw Bass): its compile() pipeline runs
    # generate_event_semaphores, which splits multi-sem waits into event-sem
    # instructions — TRN2 allows at most one sync wait per instruction.
    nc = bacc.Bacc(trn_type="TRN2")

    q_ext = nc.declare_dram_parameter("q", [BPC, T, H], f32, isOutput=False)
    p_ext = nc.declare_dram_parameter("p", [BPC, T, H], f32, isOutput=False)
    wc1_ext = nc.declare_dram_parameter("Wc1", [H, T], f32, isOutput=False)
    wc2_ext = nc.declare_dram_parameter("Wc2", [H, T], f32, isOutput=False)
    vc_ext = nc.declare_dram_parameter("vc", [T, 1], f32, isOutput=False)
    wb_ext = nc.declare_dram_parameter("Wb", [H, H], f32, isOutput=False)
    wd_ext = nc.declare_dram_parameter("Wd", [H, 1], f32, isOutput=False)
    vd_ext = nc.declare_dram_parameter("vd", [T, 1], f32, isOutput=False)
    wm_ext = nc.declare_dram_parameter("Wm", [H, 1], f32, isOutput=False)
    vm_ext = nc.declare_dram_parameter("vm", [T, 1], f32, isOutput=False)
    out_ext = nc.declare_dram_parameter("out", [4, BPC, T, H], f32, isOutput=True)

    with tile.TileContext(nc) as tc, ExitStack() as ctx:
        const = ctx.enter_context(tc.tile_pool(name="const", bufs=1))
        io = ctx.enter_context(tc.tile_pool(name="io", bufs=2))
        trans = ctx.enter_context(tc.tile_pool(name="trans", bufs=2))
        epool = ctx.enter_context(tc.tile_pool(name="epool", bufs=3))
        small = ctx.enter_context(tc.tile_pool(name="small", bufs=4))
        # PSUM budget is 8 banks total; each tag gets its own `bufs` slots of
        # one bank each: ps256(2) + sbps(2) + pstiny(2) + pso(2) = 8.
        ps256 = ctx.enter_context(tc.tile_pool(name="ps256", bufs=2, space="PSUM"))
        pssb = ctx.enter_context(tc.tile_pool(name="pssb", bufs=2, space="PSUM"))
        pstiny = ctx.enter_context(tc.tile_pool(name="pstiny", bufs=2, space="PSUM"))
        pso = ctx.enter_context(tc.tile_pool(name="pso", bufs=2, space="PSUM"))

        # ---- constants / weights (loaded once) ----
        ident = const.tile([128, 128], f32, tag="ident")
        make_identity(nc, ident)
        ones_col = const.tile([128, 1], f32, tag="ones_col")
        nc.vector.memset(ones_col, 1.0)
        ones_row = const.tile([1, 128], f32, tag="ones_row")
        nc.vector.memset(ones_row, 1.0)
        neg_ones_row = const.tile([1, 128], f32, tag="neg_ones_row")
        nc.vector.memset(neg_ones_row, -1.0)

        # vc broadcast across partitions: vc_bc[p, i] = vc[i]
        vc_bc = const.tile([128, T], f32, tag="vc_bc")
        vcf = vc_ext[:, 0]
        nc.gpsimd.dma_start(
            out=vc_bc,
            in_=bass.AP(tensor=vcf.tensor, offset=vcf.offset, ap=[[0, 128]] + vcf.ap),
        )

        wc1 = []
        wc2 = []
        wb = []
        wd = []
        wm = []
        for k in range(HK):
            t1 = const.tile([128, T], f32, tag=f"wc1_{k}", name=f"wc1_{k}")
            nc.sync.dma_start(out=t1, in_=wc1_ext[128 * k : 128 * (k + 1), :])
            wc1.append(t1)
            t2 = const.tile([128, T], f32, tag=f"wc2_{k}", name=f"wc2_{k}")
            nc.sync.dma_start(out=t2, in_=wc2_ext[128 * k : 128 * (k + 1), :])
            wc2.append(t2)
            t3 = const.tile([128, H], f32, tag=f"wb_{k}", name=f"wb_{k}")
            nc.sync.dma_start(out=t3, in_=wb_ext[128 * k : 128 * (k + 1), :])
            wb.append(t3)
            t4 = const.tile([128, 1], f32, tag=f"wd_{k}", name=f"wd_{k}")
            nc.sync.dma_start(out=t4, in_=wd_ext[128 * k : 128 * (k + 1), :])
            wd.append(t4)
            t5 = const.tile([128, 1], f32, tag=f"wm_{k}", name=f"wm_{k}")
            nc.sync.dma_start(out=t5, in_=wm_ext[128 * k : 128 * (k + 1), :])
            wm.append(t5)
        vd_c = []
        vm_c = []
        for c in range(TC):
            t6 = const.tile([128, 1], f32, tag=f"vd_{c}", name=f"vd_{c}")
            nc.sync.dma_start(out=t6, in_=vd_ext[128 * c : 128 * (c + 1), :])
            vd_c.append(t6)
            t7 = const.tile([128, 1], f32, tag=f"vm_{c}", name=f"vm_{c}")
            nc.sync.dma_start(out=t7, in_=vm_ext[128 * c : 128 * (c + 1), :])
            vm_c.append(t7)

        def copy_engine(idx):
            # alternate PSUM->SBUF copies between DVE and ACT to balance load
            return nc.vector if idx % 2 == 0 else nc.scalar

        def do_copy(idx, out, in_):
            if idx % 2 == 0:
                nc.vector.tensor_copy(out, in_)
            else:
                nc.scalar.copy(out, in_)

        # ---- per-batch body ----
        for b in range(BPC):
            # natural layout loads: qn[:, c, :] holds q[b, 128c:128c+128, :]
            qn = io.tile([128, TC, H], f32, tag="qn", name=f"qn_{b}")
            pn = io.tile([128, TC, H], f32, tag="pn", name=f"pn_{b}")
            for c in range(TC):
                nc.sync.dma_start(
                    out=qn[:, c, :], in_=q_ext[b, 128 * c : 128 * (c + 1), :]
                )
                nc.sync.dma_start(
                    out=pn[:, c, :], in_=p_ext[b, 128 * c : 128 * (c + 1), :]
                )

            # transposes: qT[:, k, :] = q[b].T[128k:128(k+1), :]  (h on partitions)
            qT = trans.tile([128, HK, T], f32, tag="qT", name=f"qT_{b}")
            pT = trans.tile([128, HK, T], f32, tag="pT", name=f"pT_{b}")
            pdT = trans.tile([128, HK, T], f32, tag="pdT", name=f"pdT_{b}")
            cidx = 0
            for k in range(HK):
                tps_q = ps256.tile([128, T], f32, tag="ps256", name=f"tpsq_{b}_{k}")
                for c in range(TC):
                    nc.tensor.transpose(
                        tps_q[:, 128 * c : 128 * (c + 1)],
                        qn[:, c, 128 * k : 128 * (k + 1)],
                        ident,
                    )
                do_copy(cidx, qT[:, k, :], tps_q)
                cidx += 1
                tps_p = ps256.tile([128, T], f32, tag="ps256", name=f"tpsp_{b}_{k}")
                for c in range(TC):
                    nc.tensor.transpose(
                        tps_p[:, 128 * c : 128 * (c + 1)],
                        pn[:, c, 128 * k : 128 * (k + 1)],
                        ident,
                    )
                do_copy(cidx, pT[:, k, :], tps_p)
                cidx += 1
                # pdT = pT * Wd[h] (per-partition scalar)
                nc.vector.tensor_scalar_mul(pdT[:, k, :], pT[:, k, :], wd[k])

            def softmax_norm_and_out(att, e):
                # e: [128, TC, T] SBUF tile of exp-scores (transposed layout).
                # Z[i] = sum_j e[j, i] via ones matmul -> [128, 1] column per
                # i-chunk; 1/Z folded into the O PSUM->SBUF evacuation.
                for ic in range(TC):
                    zcol = pstiny.tile(
                        [128, 1], f32, tag="pstiny", name=f"z_{att}_{b}_{ic}"
                    )
                    for jc in range(TC):
                        nc.tensor.matmul(
                            zcol,
                            e[:, jc, 128 * ic : 128 * (ic + 1)],
                            ones_col,
                            start=(jc == 0),
                            stop=(jc == TC - 1),
                        )
                    zrec = small.tile([128, 1], f32, tag="zrec", name=f"zr_{att}_{b}_{ic}")
                    nc.vector.reciprocal(zrec, zcol)
                    osb = epool.tile([128, H], f32, tag="osb", name=f"osb_{att}_{b}_{ic}")
                    for nh in range(H // NH):
                        ops = pso.tile(
                            [128, NH], f32, tag="pso", name=f"o_{att}_{b}_{ic}_{nh}"
                        )
                        for jc in range(TC):
                            nc.tensor.matmul(
                                ops,
                                e[:, jc, 128 * ic : 128 * (ic + 1)],
                                qn[:, jc, NH * nh : NH * (nh + 1)],
                                start=(jc == 0),
                                stop=(jc == TC - 1),
                            )
                        # normalize while evacuating PSUM (alternate DVE/ACT)
                        dst = osb[:, NH * nh : NH * (nh + 1)]
                        if (att + nh) % 2 == 0:
                            nc.vector.tensor_scalar_mul(dst, ops, zrec)
                        else:
                            nc.scalar.activation(dst, ops, AF.Copy, scale=zrec)
                    nc.sync.dma_start(
                        out=out_ext[att, b, 128 * ic : 128 * (ic + 1), :],
                        in_=osb,
                    )

            # ---------- sc (concat attention), transposed layout ----------
            e_sc = epool.tile([128, TC, T], f32, tag="e", name=f"esc_{b}")
            for jc in range(TC):
                ups = ps256.tile([128, T], f32, tag="ps256", name=f"usc_{b}_{jc}")
                for k in range(HK):
                    nc.tensor.matmul(
                        ups,
                        qT[:, k, 128 * jc : 128 * (jc + 1)],
                        wc1[k],
                        start=(k == 0),
                        stop=False,
                    )
                for k in range(HK):
                    nc.tensor.matmul(
                        ups,
                        wc2[k][:, 128 * jc : 128 * (jc + 1)],
                        pT[:, k, :],
                        start=False,
                        stop=(k == HK - 1),
                    )
                nc.scalar.activation(e_sc[:, jc, :], ups, AF.Tanh)
                nc.vector.tensor_mul(e_sc[:, jc, :], e_sc[:, jc, :], vc_bc)
                nc.scalar.activation(e_sc[:, jc, :], e_sc[:, jc, :], AF.Exp)
            softmax_norm_and_out(0, e_sc)

            # ---------- sb (bilinear attention) ----------
            # pwbT[h', i] = sum_h Wb[h, h'] * pT[h, i]
            pwbT = trans.tile([128, HK, T], f32, tag="pwbT", name=f"pwbT_{b}")
            for k2 in range(HK):
                pws = ps256.tile([128, T], f32, tag="ps256", name=f"pws_{b}_{k2}")
                for k in range(HK):
                    nc.tensor.matmul(
                        pws,
                        wb[k][:, 128 * k2 : 128 * (k2 + 1)],
                        pT[:, k, :],
                        start=(k == 0),
                        stop=(k == HK - 1),
                    )
                do_copy(k2, pwbT[:, k2, :], pws)
            e_sb = epool.tile([128, TC, T], f32, tag="e", name=f"esb_{b}")
            sbps = []
            for jc in range(TC):
                sps = pssb.tile([128, T], f32, tag="sbps", name=f"sb_{b}_{jc}")
                for k2 in range(HK):
                    nc.tensor.matmul(
                        sps,
                        qT[:, k2, 128 * jc : 128 * (jc + 1)],
                        pwbT[:, k2, :],
                        start=(k2 == 0),
                        stop=(k2 == HK - 1),
                    )
                sbps.append(sps)
            # global max over the whole [T, T] score block (valid softmax shift)
            m0 = small.tile([128, 1], f32, tag="m0", name=f"m0_{b}")
            m1 = small.tile([128, 1], f32, tag="m1", name=f"m1_{b}")
            nc.vector.reduce_max(m0, sbps[0], axis=mybir.AxisListType.X)
            nc.vector.reduce_max(m1, sbps[1], axis=mybir.AxisListType.X)
            nc.vector.tensor_max(m0, m0, m1)
            mt = pstiny.tile([1, 128], f32, tag="pstiny", name=f"mt_{b}")
            nc.tensor.transpose(mt, m0, ident)
            gneg = small.tile([1, 1], f32, tag="gneg", name=f"g_{b}")
            nc.vector.reduce_max(gneg, mt, axis=mybir.AxisListType.X)
            nc.vector.tensor_scalar_mul(gneg, gneg, -1.0)
            gnps = pstiny.tile([128, 1], f32, tag="pstiny", name=f"gnps_{b}")
            nc.tensor.matmul(gnps, ones_row, gneg, start=True, stop=True)
            gnb = small.tile([128, 1], f32, tag="gnb", name=f"gnb_{b}")
            nc.vector.tensor_copy(gnb, gnps)
            for jc in range(TC):
                nc.scalar.activation(e_sb[:, jc, :], sbps[jc], AF.Exp, bias=gnb)
            softmax_norm_and_out(1, e_sb)

            # ---------- sd (elementwise-product attention) ----------
            e_sd = epool.tile([128, TC, T], f32, tag="e", name=f"esd_{b}")
            for jc in range(TC):
                dps = ps256.tile([128, T], f32, tag="ps256", name=f"sd_{b}_{jc}")
                for k in range(HK):
                    nc.tensor.matmul(
                        dps,
                        qT[:, k, 128 * jc : 128 * (jc + 1)],
                        pdT[:, k, :],
                        start=(k == 0),
                        stop=(k == HK - 1),
                    )
                nc.scalar.activation(e_sd[:, jc, :], dps, AF.Tanh)
                nc.vector.tensor_scalar_mul(e_sd[:, jc, :], e_sd[:, jc, :], vd_c[jc])
                nc.scalar.activation(e_sd[:, jc, :], e_sd[:, jc, :], AF.Exp)
            softmax_norm_and_out(2, e_sd)

            # ---------- sm (elementwise-difference attention) ----------
            # qwm[j] = sum_h q[j,h] Wm[h]  (column, per j-chunk)
            qwm_sb = []
            for jc in range(TC):
                qws = pstiny.tile([128, 1], f32, tag="pstiny", name=f"qws_{b}_{jc}")
                for k in range(HK):
                    nc.tensor.matmul(
                        qws,
                        qT[:, k, 128 * jc : 128 * (jc + 1)],
                        wm[k],
                        start=(k == 0),
                        stop=(k == HK - 1),
                    )
                qcol = small.tile([128, 1], f32, tag="qwm", name=f"qwm_{b}_{jc}")
                nc.vector.tensor_copy(qcol, qws)
                qwm_sb.append(qcol)
            # pwm[i] = sum_h p[i,h] Wm[h]  (row), broadcast across partitions
            pws2 = pstiny.tile([1, T], f32, tag="pstiny", name=f"pwm_{b}")
            for k in range(HK):
                nc.tensor.matmul(
                    pws2, wm[k], pT[:, k, :], start=(k == 0), stop=(k == HK - 1)
                )
            pwm_row = small.tile([1, T], f32, tag="pwm_row", name=f"pwmr_{b}")
            nc.vector.tensor_copy(pwm_row, pws2)
            pwm_bc = ps256.tile([128, T], f32, tag="ps256", name=f"pwmb_{b}")
            nc.tensor.matmul(pwm_bc, ones_row, pwm_row, start=True, stop=True)
            e_sm = epool.tile([128, TC, T], f32, tag="e", name=f"esm_{b}")
            for jc in range(TC):
                # tanh(qwm[j] - pwm[i]) = Tanh(-1 * pwm_bc + qwm_col)
                nc.scalar.activation(
                    e_sm[:, jc, :], pwm_bc, AF.Tanh, bias=qwm_sb[jc], scale=-1.0
                )
                nc.vector.tensor_scalar_mul(e_sm[:, jc, :], e_sm[:, jc, :], vm_c[jc])
                nc.scalar.activation(e_sm[:, jc, :], e_sm[:, jc, :], AF.Exp)
            softmax_norm_and_out(3, e_sm)

    nc.compile()
    return nc


def _get_program():
    if "nc" not in _CACHE:
        _CACHE["nc"] = _build_program()
    return _CACHE["nc"]


def kernel(**inputs):
    global last_exec_time_ns, last_trace_dir
    from concourse.bass_utils import run_bass_kernel_spmd

    nc = _get_program()

    q = np.ascontiguousarray(np.asarray(inputs["q"], dtype=np.float32))
    p = np.ascontiguousarray(np.asarray(inputs["p"], dtype=np.float32))
    weights = {
        k: np.ascontiguousarray(np.asarray(inputs[k], dtype=np.float32))
        for k in ["Wc1", "Wc2", "vc", "Wb", "Wd", "vd", "Wm", "vm"]
    }

    in_maps = []
    for i in range(NCORES):
        m = {"q": q[i * BPC : (i + 1) * BPC], "p": p[i * BPC : (i + 1) * BPC]}
        m.update(weights)
        in_maps.append(m)

    trace = bool(int(os.environ.get("BASS_KERNEL_TRACE", "0")))
    kw = {}
    if trace:
        kw.update(trace=True)
        tmpdir = os.environ.get("BASS_KERNEL_TRACE_DIR")
        if tmpdir:
            os.makedirs(tmpdir, exist_ok=True)
            kw.update(tmpdir=tmpdir)
    res = run_bass_kernel_spmd(nc, in_maps, core_ids=list(range(NCORES)), **kw)
    last_exec_time_ns = getattr(res, "exec_time_ns", None)
    results = res.results

    outs = [np.empty((B, T, H), dtype=np.float32) for _ in range(4)]
    for i in range(NCORES):
        o = results[i]["out"]
        for a in range(4):
            outs[a][i * BPC : (i + 1) * BPC] = o[a]
    return tuple(outs)


# revision 8
# speedup vs baseline: 1.3070x; 1.3070x over previous
"""Trainium2 Bass kernel for the 4-way additive/bilinear/product/difference
attention module (B=64, T=256, H=768), data-parallel over batch across 8
NeuronCores.

Math per batch b (reference semantics):
  sc[i,j] = tanh((p@Wc2)[i,j] + (q@Wc1)[j,i]) * vc[i];  qc = softmax_j(sc) @ q
  sb[i,j] = (p@Wb@q^T)[i,j];                            qb = softmax_j(sb) @ q
  sd[i,j] = tanh(sum_h p[i,h]Wd[h]q[j,h]) * vd[j];      qd = softmax_j(sd) @ q
  sm[i,j] = tanh((q@Wm)[j] - (p@Wm)[i]) * vm[j];        qm = softmax_j(sm) @ q

Implementation notes:
- All score matrices are built TRANSPOSED (S^T[j,i], softmax axis j on
  partitions) so the attention matrix lands directly in the lhsT layout
  needed for the A@q matmul — no per-batch transposes of A.
- Matmuls run in fp16 (PSUM accumulates fp32): fp32 matmuls lower to two HW
  passes at half stream rate, ~4x slower. Validated numerics: worst
  fro-rel err 2.5e-3 vs the f32 reference (gate is 2e-2).
- exp(sb - gmax) spans e^-70 — below fp16's min subnormal — so the bilinear
  attention matrix is stored bf16 (8-bit mantissa, f32-range exponent).
  A single global max is a valid softmax shift and is numerically safe
  (measured gmax-rowmax spread < 70 < bf16/f32 underflow ~87).
- q/p are cast to fp16 during the load DMA (SWDGE cast); q^T/p^T come from
  hardware DMA-transpose (2-byte dtypes only) — no TensorE transposes.
- Softmax denominators via ones-vector matmuls; 1/Z is folded into the
  PSUM->SBUF output evacuation as a per-partition tensor_scalar multiply.
"""

import os

import numpy as np

B, T, H = 64, 256, 768
NCORES = 8
BPC = B // NCORES  # batches per core
HK = H // 128  # 6 h-chunks
TC = T // 128  # 2 t-chunks
NH = 384  # output free-dim half (PSUM bank limit: 512 f32)

_CACHE = {}

# set by kernel() when BASS_KERNEL_TRACE=1 (read by test harness)
last_exec_time_ns = None
last_trace_dir = None


def _build_program():
    from contextlib import ExitStack

    import concourse.bass as bass
    import concourse.tile as tile
    from concourse import bacc, mybir
    from concourse.masks import make_identity

    f32 = mybir.dt.float32
    f16 = mybir.dt.float16
    bf16 = mybir.dt.bfloat16
    AF = mybir.ActivationFunctionType

    # Bacc (not raw Bass): its compile() pipeline runs
    # generate_event_semaphores, which splits multi-sem waits into event-sem
    # instructions — TRN2 allows at most one sync wait per instruction.
    nc = bacc.Bacc(trn_type="TRN2")

    q_ext = nc.declare_dram_parameter("q", [BPC, T, H], f32, isOutput=False)
    p_ext = nc.declare_dram_parameter("p", [BPC, T, H], f32, isOutput=False)
    wc1_ext = nc.declare_dram_parameter("Wc1", [H, T], f32, isOutput=False)
    wc2_ext = nc.declare_dram_parameter("Wc2", [H, T], f32, isOutput=False)
    vc_ext = nc.declare_dram_parameter("vc", [T, 1], f32, isOutput=False)
    wb_ext = nc.declare_dram_parameter("Wb", [H, H], f32, isOutput=False)
    wd_ext = nc.declare_dram_parameter("Wd", [H, 1], f32, isOutput=False)
    vd_ext = nc.declare_dram_parameter("vd", [T, 1], f32, isOutput=False)
    wm_ext = nc.declare_dram_parameter("Wm", [H, 1], f32, isOutput=False)
    vm_ext = nc.declare_dram_parameter("vm", [T, 1], f32, isOutput=False)
    out_ext = nc.declare_dram_parameter("out", [4, BPC, T, H], f32, isOutput=True)

    with tile.TileContext(nc) as tc, ExitStack() as ctx:
        const = ctx.enter_context(tc.tile_pool(name="const", bufs=1))
        io = ctx.enter_context(tc.tile_pool(name="io", bufs=2))
        trans = ctx.enter_context(tc.tile_pool(name="trans", bufs=2))
        epool = ctx.enter_context(tc.tile_pool(name="epool", bufs=3))
        small = ctx.enter_context(tc.tile_pool(name="small", bufs=4))
        # PSUM budget is 8 banks; each tag gets its own `bufs` slots of one
        # bank: ps256(2) + sbps(2) + pstiny(2) + pso(2) = 8.
        ps256 = ctx.enter_context(tc.tile_pool(name="ps256", bufs=2, space="PSUM"))
        pssb = ctx.enter_context(tc.tile_pool(name="pssb", bufs=2, space="PSUM"))
        pstiny = ctx.enter_context(tc.tile_pool(name="pstiny", bufs=2, space="PSUM"))
        pso = ctx.enter_context(tc.tile_pool(name="pso", bufs=2, space="PSUM"))

        # ---- constants / weights (loaded once, cast to fp16 in the DMA) ----
        ident = const.tile([128, 128], f16, tag="ident")
        make_identity(nc, ident)
        ones_col = const.tile([128, 1], f16, tag="ones_col")
        nc.vector.memset(ones_col, 1.0)
        ones_col_bf = const.tile([128, 1], bf16, tag="ones_col_bf")
        nc.vector.memset(ones_col_bf, 1.0)
        ones_row = const.tile([1, 128], f16, tag="ones_row")
        nc.vector.memset(ones_row, 1.0)

        # vc broadcast across partitions: vc_bc[p, i] = vc[i]  (kept f32)
        vc_bc = const.tile([128, T], f32, tag="vc_bc")
        vcf = vc_ext[:, 0]
        nc.gpsimd.dma_start(
            out=vc_bc,
            in_=bass.AP(tensor=vcf.tensor, offset=vcf.offset, ap=[[0, 128]] + vcf.ap),
        )

        wc1 = []
        wc2 = []
        wb = []
        wd = []
        wm = []
        for k in range(HK):
            t1 = const.tile([128, T], f16, tag=f"wc1_{k}", name=f"wc1_{k}")
            nc.gpsimd.dma_start(out=t1, in_=wc1_ext[128 * k : 128 * (k + 1), :])
            wc1.append(t1)
            t2 = const.tile([128, T], f16, tag=f"wc2_{k}", name=f"wc2_{k}")
            nc.gpsimd.dma_start(out=t2, in_=wc2_ext[128 * k : 128 * (k + 1), :])
            wc2.append(t2)
            t3 = const.tile([128, H], f16, tag=f"wb_{k}", name=f"wb_{k}")
            nc.gpsimd.dma_start(out=t3, in_=wb_ext[128 * k : 128 * (k + 1), :])
            wb.append(t3)
            t4 = const.tile([128, 1], f32, tag=f"wd_{k}", name=f"wd_{k}")
            nc.sync.dma_start(out=t4, in_=wd_ext[128 * k : 128 * (k + 1), :])
            wd.append(t4)
            t5 = const.tile([128, 1], f16, tag=f"wm_{k}", name=f"wm_{k}")
            nc.gpsimd.dma_start(out=t5, in_=wm_ext[128 * k : 128 * (k + 1), :])
            wm.append(t5)
        vd_c = []
        vm_c = []
        for c in range(TC):
            t6 = const.tile([128, 1], f32, tag=f"vd_{c}", name=f"vd_{c}")
            nc.sync.dma_start(out=t6, in_=vd_ext[128 * c : 128 * (c + 1), :])
            vd_c.append(t6)
            t7 = const.tile([128, 1], f32, tag=f"vm_{c}", name=f"vm_{c}")
            nc.sync.dma_start(out=t7, in_=vm_ext[128 * c : 128 * (c + 1), :])
            vm_c.append(t7)

        # ---- per-batch body ----
        for b in range(BPC):
            # fp16 natural-layout loads (cast during DMA): qn[:, c, :] holds
            # q[b, 128c:128(c+1), :]
            qn = io.tile([128, TC, H], f16, tag="qn", name=f"qn_{b}")
            pn = io.tile([128, TC, H], f16, tag="pn", name=f"pn_{b}")
            for c in range(TC):
                nc.gpsimd.dma_start(
                    out=qn[:, c, :], in_=q_ext[b, 128 * c : 128 * (c + 1), :]
                )
                nc.gpsimd.dma_start(
                    out=pn[:, c, :], in_=p_ext[b, 128 * c : 128 * (c + 1), :]
                )
            # bf16 copy of q for the bilinear attention's A@q matmul
            qn_bf = io.tile([128, TC, H], bf16, tag="qn_bf", name=f"qnbf_{b}")
            for c in range(TC):
                nc.scalar.copy(qn_bf[:, c, :], qn[:, c, :])

            # transposes via hardware DMA-transpose (SBUF->SBUF, fp16)
            qT = trans.tile([128, HK, T], f16, tag="qT", name=f"qT_{b}")
            pT = trans.tile([128, HK, T], f16, tag="pT", name=f"pT_{b}")
            pdT = trans.tile([128, HK, T], f16, tag="pdT", name=f"pdT_{b}")
            for k in range(HK):
                for c in range(TC):
                    nc.sync.dma_start(
                        out=qT[:, k, 128 * c : 128 * (c + 1)],
                        in_=qn[:, c, 128 * k : 128 * (k + 1)],
                        transpose=True,
                    )
                    nc.sync.dma_start(
                        out=pT[:, k, 128 * c : 128 * (c + 1)],
                        in_=pn[:, c, 128 * k : 128 * (k + 1)],
                        transpose=True,
                    )
                # pdT = pT * Wd[h] (per-partition scalar)
                nc.vector.tensor_scalar_mul(pdT[:, k, :], pT[:, k, :], wd[k])

            def softmax_norm_and_out(att, e, rhs_qn, ones):
                # e: [128, TC, T] SBUF exp-scores (transposed layout).
                # Z[i] = sum_j e[j, i] via ones matmul -> [128,1] per i-chunk;
                # 1/Z folded into the O PSUM->SBUF evacuation.
                for ic in range(TC):
                    zcol = pstiny.tile(
                        [128, 1], f32, tag="pstiny", name=f"z_{att}_{b}_{ic}"
                    )
                    for jc in range(TC):
                        nc.tensor.matmul(
                            zcol,
                            e[:, jc, 128 * ic : 128 * (ic + 1)],
                            ones,
                            start=(jc == 0),
                            stop=(jc == TC - 1),
                        )
                    zrec = small.tile(
                        [128, 1], f32, tag="zrec", name=f"zr_{att}_{b}_{ic}"
                    )
                    nc.vector.reciprocal(zrec, zcol)
                    osb = epool.tile(
                        [128, H], f32, tag="osb", name=f"osb_{att}_{b}_{ic}"
                    )
                    for nh in range(H // NH):
                        ops = pso.tile(
                            [128, NH], f32, tag="pso", name=f"o_{att}_{b}_{ic}_{nh}"
                        )
                        for jc in range(TC):
                            nc.tensor.matmul(
                                ops,
                                e[:, jc, 128 * ic : 128 * (ic + 1)],
                                rhs_qn[:, jc, NH * nh : NH * (nh + 1)],
                                start=(jc == 0),
                                stop=(jc == TC - 1),
                            )
                        # normalize while evacuating PSUM (alternate DVE/ACT)
                        dst = osb[:, NH * nh : NH * (nh + 1)]
                        if (att + nh) % 2 == 0:
                            nc.vector.tensor_scalar_mul(dst, ops, zrec)
                        else:
                            nc.scalar.activation(dst, ops, AF.Copy, scale=zrec)
                    nc.sync.dma_start(
                        out=out_ext[att, b, 128 * ic : 128 * (ic + 1), :],
                        in_=osb,
                    )

            # ---------- sc (concat attention), transposed layout ----------
            e_sc = epool.tile([128, TC, T], f16, tag="e", name=f"esc_{b}")
            for jc in range(TC):
                ups = ps256.tile([128, T], f32, tag="ps256", name=f"usc_{b}_{jc}")
                for k in range(HK):
                    nc.tensor.matmul(
                        ups,
                        qT[:, k, 128 * jc : 128 * (jc + 1)],
                        wc1[k],
                        start=(k == 0),
                        stop=False,
                    )
                for k in range(HK):
                    nc.tensor.matmul(
                        ups,
                        wc2[k][:, 128 * jc : 128 * (jc + 1)],
                        pT[:, k, :],
                        start=False,
                        stop=(k == HK - 1),
                    )
                tmp = epool.tile([128, T], f32, tag="tmp", name=f"tsc_{b}_{jc}")
                nc.scalar.activation(tmp, ups, AF.Tanh)
                nc.vector.tensor_mul(tmp, tmp, vc_bc)
                nc.scalar.activation(e_sc[:, jc, :], tmp, AF.Exp)
            softmax_norm_and_out(0, e_sc, qn, ones_col)

            # ---------- sb (bilinear attention) ----------
            # pwbT[h', i] = sum_h Wb[h, h'] * pT[h, i]
            pwbT = trans.tile([128, HK, T], f16, tag="pwbT", name=f"pwbT_{b}")
            for k2 in range(HK):
                pws = ps256.tile([128, T], f32, tag="ps256", name=f"pws_{b}_{k2}")
                for k in range(HK):
                    nc.tensor.matmul(
                        pws,
                        wb[k][:, 128 * k2 : 128 * (k2 + 1)],
                        pT[:, k, :],
                        start=(k == 0),
                        stop=(k == HK - 1),
                    )
                if k2 % 2 == 0:
                    nc.vector.tensor_copy(pwbT[:, k2, :], pws)
                else:
                    nc.scalar.copy(pwbT[:, k2, :], pws)
            e_sb = epool.tile([128, TC, T], bf16, tag="e_bf", name=f"esb_{b}")
            sbps = []
            for jc in range(TC):
                sps = pssb.tile([128, T], f32, tag="sbps", name=f"sb_{b}_{jc}")
                for k2 in range(HK):
                    nc.tensor.matmul(
                        sps,
                        qT[:, k2, 128 * jc : 128 * (jc + 1)],
                        pwbT[:, k2, :],
                        start=(k2 == 0),
                        stop=(k2 == HK - 1),
                    )
                sbps.append(sps)
            # global max over the whole [T, T] score block (valid softmax shift)
            m0 = small.tile([128, 1], f16, tag="m0", name=f"m0_{b}")
            m1 = small.tile([128, 1], f16, tag="m1", name=f"m1_{b}")
            nc.vector.reduce_max(m0, sbps[0], axis=mybir.AxisListType.X)
            nc.vector.reduce_max(m1, sbps[1], axis=mybir.AxisListType.X)
            nc.vector.tensor_max(m0, m0, m1)
            mt = pstiny.tile([1, 128], f16, tag="pstiny", name=f"mt_{b}")
            nc.tensor.transpose(mt, m0, ident)
            gneg = small.tile([1, 1], f16, tag="gneg", name=f"g_{b}")
            nc.vector.reduce_max(gneg, mt, axis=mybir.AxisListType.X)
            nc.vector.tensor_scalar_mul(gneg, gneg, -1.0)
            gnps = pstiny.tile([128, 1], f32, tag="pstiny", name=f"gnps_{b}")
            nc.tensor.matmul(gnps, ones_row, gneg, start=True, stop=True)
            gnb = small.tile([128, 1], f32, tag="gnb", name=f"gnb_{b}")
            nc.vector.tensor_copy(gnb, gnps)
            for jc in range(TC):
                nc.scalar.activation(e_sb[:, jc, :], sbps[jc], AF.Exp, bias=gnb)
            softmax_norm_and_out(1, e_sb, qn_bf, ones_col_bf)

            # ---------- sd (elementwise-product attention) ----------
            e_sd = epool.tile([128, TC, T], f16, tag="e", name=f"esd_{b}")
            for jc in range(TC):
                dps = ps256.tile([128, T], f32, tag="ps256", name=f"sd_{b}_{jc}")
                for k in range(HK):
                    nc.tensor.matmul(
                        dps,
                        qT[:, k, 128 * jc : 128 * (jc + 1)],
                        pdT[:, k, :],
                        start=(k == 0),
                        stop=(k == HK - 1),
                    )
                tmp = epool.tile([128, T], f32, tag="tmp", name=f"tsd_{b}_{jc}")
                nc.scalar.activation(tmp, dps, AF.Tanh)
                nc.vector.tensor_scalar_mul(tmp, tmp, vd_c[jc])
                nc.scalar.activation(e_sd[:, jc, :], tmp, AF.Exp)
            softmax_norm_and_out(2, e_sd, qn, ones_col)

            # ---------- sm (elementwise-difference attention) ----------
            # qwm[j] = sum_h q[j,h] Wm[h]  (column, per j-chunk)
            qwm_sb = []
            for jc in range(TC):
                qws = pstiny.tile([128, 1], f32, tag="pstiny", name=f"qws_{b}_{jc}")
                for k in range(HK):
                    nc.tensor.matmul(
                        qws,
                        qT[:, k, 128 * jc : 128 * (jc + 1)],
                        wm[k],
                        start=(k == 0),
                        stop=(k == HK - 1),
                    )
                qcol = small.tile([128, 1], f32, tag="qwm", name=f"qwm_{b}_{jc}")
                nc.vector.tensor_copy(qcol, qws)
                qwm_sb.append(qcol)
            # pwm[i] = sum_h p[i,h] Wm[h]  (row), broadcast across partitions
            pws2 = pstiny.tile([1, T], f32, tag="pstiny", name=f"pwm_{b}")
            for k in range(HK):
                nc.tensor.matmul(
                    pws2, wm[k], pT[:, k, :], start=(k == 0), stop=(k == HK - 1)
                )
            pwm_row = small.tile([1, T], f16, tag="pwm_row", name=f"pwmr_{b}")
            nc.vector.tensor_copy(pwm_row, pws2)
            pwm_bc = ps256.tile([128, T], f32, tag="ps256", name=f"pwmb_{b}")
            nc.tensor.matmul(pwm_bc, ones_row, pwm_row, start=True, stop=True)
            e_sm = epool.tile([128, TC, T], f16, tag="e", name=f"esm_{b}")
            for jc in range(TC):
                # tanh(qwm[j] - pwm[i]) = Tanh(-1 * pwm_bc + qwm_col)
                tmp = epool.tile([128, T], f32, tag="tmp", name=f"tsm_{b}_{jc}")
                nc.scalar.activation(
                    tmp, pwm_bc, AF.Tanh, bias=qwm_sb[jc], scale=-1.0
                )
                nc.vector.tensor_scalar_mul(tmp, tmp, vm_c[jc])
                nc.scalar.activation(e_sm[:, jc, :], tmp, AF.Exp)
            softmax_norm_and_out(3, e_sm, qn, ones_col)

    nc.compile()
    return nc


def _get_program():
    if "nc" not in _CACHE:
        _CACHE["nc"] = _build_program()
    return _CACHE["nc"]


def kernel(**inputs):
    global last_exec_time_ns, last_trace_dir
    from concourse.bass_utils import run_bass_kernel_spmd

    nc = _get_program()

    q = np.ascontiguousarray(np.asarray(inputs["q"], dtype=np.float32))
    p = np.ascontiguousarray(np.asarray(inputs["p"], dtype=np.float32))
    weights = {
        k: np.ascontiguousarray(np.asarray(inputs[k], dtype=np.float32))
        for k in ["Wc1", "Wc2", "vc", "Wb", "Wd", "vd", "Wm", "vm"]
    }

    in_maps = []
    for i in range(NCORES):
        m = {"q": q[i * BPC : (i + 1) * BPC], "p": p[i * BPC : (i + 1) * BPC]}
        m.update(weights)
        in_maps.append(m)

    trace = bool(int(os.environ.get("BASS_KERNEL_TRACE", "0")))
    kw = {}
    if trace:
        kw.update(trace=True)
        tmpdir = os.environ.get("BASS_KERNEL_TRACE_DIR")
        if tmpdir:
            os.makedirs(tmpdir, exist_ok=True)
            kw.update(tmpdir=tmpdir)
    res = run_bass_kernel_spmd(nc, in_maps, core_ids=list(range(NCORES)), **kw)
    last_exec_time_ns = getattr(res, "exec_time_ns", None)
    results = res.results

    outs = [np.empty((B, T, H), dtype=np.float32) for _ in range(4)]
    for i in range(NCORES):
        o = results[i]["out"]
        for a in range(4):
            outs[a][i * BPC : (i + 1) * BPC] = o[a]
    return tuple(outs)


# revision 14
# speedup vs baseline: 2.5534x; 1.9536x over previous
"""Trainium2 Bass kernel for the 4-way additive/bilinear/product/difference
attention module (B=64, T=256, H=768), data-parallel over batch across 8
NeuronCores.

Math per batch b (reference semantics):
  sc[i,j] = tanh((p@Wc2)[i,j] + (q@Wc1)[j,i]) * vc[i];  qc = softmax_j(sc) @ q
  sb[i,j] = (p@Wb@q^T)[i,j];                            qb = softmax_j(sb) @ q
  sd[i,j] = tanh(sum_h p[i,h]Wd[h]q[j,h]) * vd[j];      qd = softmax_j(sd) @ q
  sm[i,j] = tanh((q@Wm)[j] - (p@Wm)[i]) * vm[j];        qm = softmax_j(sm) @ q

Implementation notes:
- All score matrices are built TRANSPOSED (S^T[j,i], softmax axis j on
  partitions) so the attention matrix lands directly in the lhsT layout
  needed for the A@q matmul — no per-batch transposes of A.
- Matmuls run in fp16 (PSUM accumulates fp32): fp32 matmuls lower to two HW
  passes at half stream rate, ~4x slower. Validated numerics: worst
  fro-rel err 2.5e-3 vs the f32 reference (gate is 2e-2).
- exp(sb - gmax) spans e^-70 — below fp16's min subnormal — so the bilinear
  attention matrix is stored bf16 (8-bit mantissa, f32-range exponent).
  A single global max is a valid softmax shift and is numerically safe
  (measured gmax-rowmax spread < 70 < bf16/f32 underflow ~87).
- q/p are cast to fp16 during the load DMA (SWDGE cast); q^T/p^T come from
  TensorE transpose-mode matmuls (DMA-transpose measured 1.2us per 128x128
  block and serialized the HWDGE queue; PE does it in ~0.1us and the dense
  stream keeps the HAM clock-gate warm).
- Softmax denominators via ones-vector matmuls; 1/Z is folded into the
  PSUM->SBUF output evacuation as a per-partition tensor_scalar multiply.
"""

import os

import numpy as np

B, T, H = 64, 256, 768
NCORES = 8
BPC = B // NCORES  # batches per core
HK = H // 128  # 6 h-chunks
TC = T // 128  # 2 t-chunks
NH = 384  # output free-dim half (PSUM bank limit: 512 f32)

_CACHE = {}

# set by kernel() when BASS_KERNEL_TRACE=1 (read by test harness)
last_exec_time_ns = None
last_trace_dir = None


def _build_program():
    from contextlib import ExitStack

    import concourse.bass as bass
    import concourse.tile as tile
    from concourse import bacc, mybir
    from concourse.masks import make_identity

    f32 = mybir.dt.float32
    f16 = mybir.dt.float16
    bf16 = mybir.dt.bfloat16
    AF = mybir.ActivationFunctionType

    # Bacc (not raw Bass): its compile() pipeline runs
    # generate_event_semaphores, which splits multi-sem waits into event-sem
    # instructions — TRN2 allows at most one sync wait per instruction.
    nc = bacc.Bacc(trn_type="TRN2")

    q_ext = nc.declare_dram_parameter("q", [BPC, T, H], f32, isOutput=False)
    p_ext = nc.declare_dram_parameter("p", [BPC, T, H], f32, isOutput=False)
    wc1_ext = nc.declare_dram_parameter("Wc1", [H, T], f32, isOutput=False)
    wc2_ext = nc.declare_dram_parameter("Wc2", [H, T], f32, isOutput=False)
    vc_ext = nc.declare_dram_parameter("vc", [T, 1], f32, isOutput=False)
    wb_ext = nc.declare_dram_parameter("Wb", [H, H], f32, isOutput=False)
    wd_ext = nc.declare_dram_parameter("Wd", [H, 1], f32, isOutput=False)
    vd_ext = nc.declare_dram_parameter("vd", [T, 1], f32, isOutput=False)
    wm_ext = nc.declare_dram_parameter("Wm", [H, 1], f32, isOutput=False)
    vm_ext = nc.declare_dram_parameter("vm", [T, 1], f32, isOutput=False)
    out_ext = nc.declare_dram_parameter("out", [4, BPC, T, H], f32, isOutput=True)

    with tile.TileContext(nc) as tc, ExitStack() as ctx:
        const = ctx.enter_context(tc.tile_pool(name="const", bufs=1))
        io = ctx.enter_context(tc.tile_pool(name="io", bufs=3))
        trans = ctx.enter_context(tc.tile_pool(name="trans", bufs=2))
        epool = ctx.enter_context(tc.tile_pool(name="epool", bufs=3))
        small = ctx.enter_context(tc.tile_pool(name="small", bufs=4))
        # PSUM budget is 8 banks; each tag gets its own `bufs` slots of one
        # bank: ps256(2) + tr(2) + pstiny(2) + pso(2) = 8.
        ps256 = ctx.enter_context(tc.tile_pool(name="ps256", bufs=2, space="PSUM"))
        pstr = ctx.enter_context(tc.tile_pool(name="pstr", bufs=2, space="PSUM"))
        pstiny = ctx.enter_context(tc.tile_pool(name="pstiny", bufs=2, space="PSUM"))
        pso = ctx.enter_context(tc.tile_pool(name="pso", bufs=2, space="PSUM"))

        # ---- constants / weights (loaded once, cast to fp16 in the DMA) ----
        ident = const.tile([128, 128], f16, tag="ident")
        make_identity(nc, ident)
        ones_col = const.tile([128, 1], f16, tag="ones_col")
        nc.vector.memset(ones_col, 1.0)
        ones_col_bf = const.tile([128, 1], bf16, tag="ones_col_bf")
        nc.vector.memset(ones_col_bf, 1.0)
        ones_row = const.tile([1, 128], f16, tag="ones_row")
        nc.vector.memset(ones_row, 1.0)

        # vc broadcast across partitions: vc_bc[p, i] = vc[i]  (kept f32)
        vc_bc = const.tile([128, T], f32, tag="vc_bc")
        vcf = vc_ext[:, 0]
        nc.gpsimd.dma_start(
            out=vc_bc,
            in_=bass.AP(tensor=vcf.tensor, offset=vcf.offset, ap=[[0, 128]] + vcf.ap),
        )

        wc1 = []
        wc2 = []
        wb = []
        wd = []
        wm = []
        for k in range(HK):
            t1 = const.tile([128, T], f16, tag=f"wc1_{k}", name=f"wc1_{k}")
            nc.gpsimd.dma_start(out=t1, in_=wc1_ext[128 * k : 128 * (k + 1), :])
            wc1.append(t1)
            t2 = const.tile([128, T], f16, tag=f"wc2_{k}", name=f"wc2_{k}")
            nc.gpsimd.dma_start(out=t2, in_=wc2_ext[128 * k : 128 * (k + 1), :])
            wc2.append(t2)
            t3 = const.tile([128, H], f16, tag=f"wb_{k}", name=f"wb_{k}")
            nc.gpsimd.dma_start(out=t3, in_=wb_ext[128 * k : 128 * (k + 1), :])
            wb.append(t3)
            t4 = const.tile([128, 1], f32, tag=f"wd_{k}", name=f"wd_{k}")
            nc.sync.dma_start(out=t4, in_=wd_ext[128 * k : 128 * (k + 1), :])
            wd.append(t4)
            t5 = const.tile([128, 1], f16, tag=f"wm_{k}", name=f"wm_{k}")
            nc.gpsimd.dma_start(out=t5, in_=wm_ext[128 * k : 128 * (k + 1), :])
            wm.append(t5)
        vd_c = []
        vm_c = []
        for c in range(TC):
            t6 = const.tile([128, 1], f32, tag=f"vd_{c}", name=f"vd_{c}")
            nc.sync.dma_start(out=t6, in_=vd_ext[128 * c : 128 * (c + 1), :])
            vd_c.append(t6)
            t7 = const.tile([128, 1], f32, tag=f"vm_{c}", name=f"vm_{c}")
            nc.sync.dma_start(out=t7, in_=vm_ext[128 * c : 128 * (c + 1), :])
            vm_c.append(t7)

        # ---- per-batch body ----
        for b in range(BPC):
            # fp16 natural-layout loads (cast during DMA): qn[:, c, :] holds
            # q[b, 128c:128(c+1), :]
            qn = io.tile([128, TC, H], f16, tag="qn", name=f"qn_{b}")
            pn = io.tile([128, TC, H], f16, tag="pn", name=f"pn_{b}")
            for c in range(TC):
                nc.gpsimd.dma_start(
                    out=qn[:, c, :], in_=q_ext[b, 128 * c : 128 * (c + 1), :]
                )
                nc.gpsimd.dma_start(
                    out=pn[:, c, :], in_=p_ext[b, 128 * c : 128 * (c + 1), :]
                )
            # bf16 copy of q for the bilinear attention's A@q matmul
            qn_bf = io.tile([128, TC, H], bf16, tag="qn_bf", name=f"qnbf_{b}")
            for c in range(TC):
                nc.scalar.copy(qn_bf[:, c, :], qn[:, c, :])

            # transposes on TensorE (fp16 transpose-mode matmul), PSUM staging
            qT = trans.tile([128, HK, T], f16, tag="qT", name=f"qT_{b}")
            pT = trans.tile([128, HK, T], f16, tag="pT", name=f"pT_{b}")
            pdT = trans.tile([128, HK, T], f16, tag="pdT", name=f"pdT_{b}")
            cidx = 0
            for k in range(HK):
                tq = pstr.tile([128, T], f16, tag="pstr", name=f"tq_{b}_{k}")
                for c in range(TC):
                    nc.tensor.transpose(
                        tq[:, 128 * c : 128 * (c + 1)],
                        qn[:, c, 128 * k : 128 * (k + 1)],
                        ident,
                    )
                if cidx % 2 == 0:
                    nc.vector.tensor_copy(qT[:, k, :], tq)
                else:
                    nc.scalar.copy(qT[:, k, :], tq)
                cidx += 1
                tp = pstr.tile([128, T], f16, tag="pstr", name=f"tp_{b}_{k}")
                for c in range(TC):
                    nc.tensor.transpose(
                        tp[:, 128 * c : 128 * (c + 1)],
                        pn[:, c, 128 * k : 128 * (k + 1)],
                        ident,
                    )
                if cidx % 2 == 0:
                    nc.vector.tensor_copy(pT[:, k, :], tp)
                else:
                    nc.scalar.copy(pT[:, k, :], tp)
                cidx += 1
                # pdT = pT * Wd[h] (per-partition scalar)
                nc.vector.tensor_scalar_mul(pdT[:, k, :], pT[:, k, :], wd[k])

            def softmax_norm_and_out(att, e, rhs_qn, ones):
                # e: [128, TC, T] SBUF exp-scores (transposed layout).
                # Z[i] = sum_j e[j, i] via ones matmul -> [128,1] per i-chunk;
                # 1/Z folded into the O PSUM->SBUF evacuation.
                for ic in range(TC):
                    zcol = pstiny.tile(
                        [128, 1], f32, tag="pstiny", name=f"z_{att}_{b}_{ic}"
                    )
                    for jc in range(TC):
                        nc.tensor.matmul(
                            zcol,
                            e[:, jc, 128 * ic : 128 * (ic + 1)],
                            ones,
                            start=(jc == 0),
                            stop=(jc == TC - 1),
                        )
                    zrec = small.tile(
                        [128, 1], f32, tag="zrec", name=f"zr_{att}_{b}_{ic}"
                    )
                    nc.vector.reciprocal(zrec, zcol)
                    osb = epool.tile(
                        [128, H], f32, tag="osb", name=f"osb_{att}_{b}_{ic}"
                    )
                    for nh in range(H // NH):
                        ops = pso.tile(
                            [128, NH], f32, tag="pso", name=f"o_{att}_{b}_{ic}_{nh}"
                        )
                        for jc in range(TC):
                            nc.tensor.matmul(
                                ops,
                                e[:, jc, 128 * ic : 128 * (ic + 1)],
                                rhs_qn[:, jc, NH * nh : NH * (nh + 1)],
                                start=(jc == 0),
                                stop=(jc == TC - 1),
                            )
                        # normalize while evacuating PSUM (alternate DVE/ACT)
                        dst = osb[:, NH * nh : NH * (nh + 1)]
                        if (att + nh) % 2 == 0:
                            nc.vector.tensor_scalar_mul(dst, ops, zrec)
                        else:
                            nc.scalar.activation(dst, ops, AF.Copy, scale=zrec)
                    # alternate the two HWDGE rings (SP / ACT sequencer)
                    dma_eng = nc.sync if (att + ic) % 2 == 0 else nc.scalar
                    dma_eng.dma_start(
                        out=out_ext[att, b, 128 * ic : 128 * (ic + 1), :],
                        in_=osb,
                    )

            # ---------- sc (concat attention), transposed layout ----------
            e_sc = epool.tile([128, TC, T], f16, tag="e", name=f"esc_{b}")
            for jc in range(TC):
                ups = ps256.tile([128, T], f32, tag="ps256", name=f"usc_{b}_{jc}")
                for k in range(HK):
                    nc.tensor.matmul(
                        ups,
                        qT[:, k, 128 * jc : 128 * (jc + 1)],
                        wc1[k],
                        start=(k == 0),
                        stop=False,
                    )
                for k in range(HK):
                    nc.tensor.matmul(
                        ups,
                        wc2[k][:, 128 * jc : 128 * (jc + 1)],
                        pT[:, k, :],
                        start=False,
                        stop=(k == HK - 1),
                    )
                tmp = epool.tile([128, T], f32, tag="tmp", name=f"tsc_{b}_{jc}")
                nc.scalar.activation(tmp, ups, AF.Tanh)
                nc.vector.tensor_mul(tmp, tmp, vc_bc)
                nc.scalar.activation(e_sc[:, jc, :], tmp, AF.Exp)
            softmax_norm_and_out(0, e_sc, qn, ones_col)

            # ---------- sb (bilinear attention) ----------
            # pwbT[h', i] = sum_h Wb[h, h'] * pT[h, i]
            pwbT = trans.tile([128, HK, T], f16, tag="pwbT", name=f"pwbT_{b}")
            for k2 in range(HK):
                pws = ps256.tile([128, T], f32, tag="ps256", name=f"pws_{b}_{k2}")
                for k in range(HK):
                    nc.tensor.matmul(
                        pws,
                        wb[k][:, 128 * k2 : 128 * (k2 + 1)],
                        pT[:, k, :],
                        start=(k == 0),
                        stop=(k == HK - 1),
                    )
                if k2 % 2 == 0:
                    nc.vector.tensor_copy(pwbT[:, k2, :], pws)
                else:
                    nc.scalar.copy(pwbT[:, k2, :], pws)
            e_sb = epool.tile([128, TC, T], bf16, tag="e_bf", name=f"esb_{b}")
            # stage sb scores in SBUF f32 (keeps PSUM bank budget at 8)
            sbsb = epool.tile([128, TC, T], f32, tag="sb_sb", name=f"sbsb_{b}")
            for jc in range(TC):
                sps = ps256.tile([128, T], f32, tag="ps256", name=f"sb_{b}_{jc}")
                for k2 in range(HK):
                    nc.tensor.matmul(
                        sps,
                        qT[:, k2, 128 * jc : 128 * (jc + 1)],
                        pwbT[:, k2, :],
                        start=(k2 == 0),
                        stop=(k2 == HK - 1),
                    )
                if jc % 2 == 0:
                    nc.vector.tensor_copy(sbsb[:, jc, :], sps)
                else:
                    nc.scalar.copy(sbsb[:, jc, :], sps)
            sbps = [sbsb[:, 0, :], sbsb[:, 1, :]]
            # global max over the whole [T, T] score block (valid softmax shift)
            m0 = small.tile([128, 1], f16, tag="m0", name=f"m0_{b}")
            m1 = small.tile([128, 1], f16, tag="m1", name=f"m1_{b}")
            nc.vector.reduce_max(m0, sbps[0], axis=mybir.AxisListType.X)
            nc.vector.reduce_max(m1, sbps[1], axis=mybir.AxisListType.X)
            nc.vector.tensor_max(m0, m0, m1)
            mt = pstiny.tile([1, 128], f16, tag="pstiny", name=f"mt_{b}")
            nc.tensor.transpose(mt, m0, ident)
            gneg = small.tile([1, 1], f16, tag="gneg", name=f"g_{b}")
            nc.vector.reduce_max(gneg, mt, axis=mybir.AxisListType.X)
            nc.vector.tensor_scalar_mul(gneg, gneg, -1.0)
            gnps = pstiny.tile([128, 1], f32, tag="pstiny", name=f"gnps_{b}")
            nc.tensor.matmul(gnps, ones_row, gneg, start=True, stop=True)
            gnb = small.tile([128, 1], f32, tag="gnb", name=f"gnb_{b}")
            nc.vector.tensor_copy(gnb, gnps)
            for jc in range(TC):
                nc.scalar.activation(e_sb[:, jc, :], sbps[jc], AF.Exp, bias=gnb)
            softmax_norm_and_out(1, e_sb, qn_bf, ones_col_bf)

            # ---------- sd (elementwise-product attention) ----------
            e_sd = epool.tile([128, TC, T], f16, tag="e", name=f"esd_{b}")
            for jc in range(TC):
                dps = ps256.tile([128, T], f32, tag="ps256", name=f"sd_{b}_{jc}")
                for k in range(HK):
                    nc.tensor.matmul(
                        dps,
                        qT[:, k, 128 * jc : 128 * (jc + 1)],
                        pdT[:, k, :],
                        start=(k == 0),
                        stop=(k == HK - 1),
                    )
                tmp = epool.tile([128, T], f32, tag="tmp", name=f"tsd_{b}_{jc}")
                nc.scalar.activation(tmp, dps, AF.Tanh)
                nc.vector.tensor_scalar_mul(tmp, tmp, vd_c[jc])
                nc.scalar.activation(e_sd[:, jc, :], tmp, AF.Exp)
            softmax_norm_and_out(2, e_sd, qn, ones_col)

            # ---------- sm (elementwise-difference attention) ----------
            # qwm[j] = sum_h q[j,h] Wm[h]  (column, per j-chunk)
            qwm_sb = []
            for jc in range(TC):
                qws = pstiny.tile([128, 1], f32, tag="pstiny", name=f"qws_{b}_{jc}")
                for k in range(HK):
                    nc.tensor.matmul(
                        qws,
                        qT[:, k, 128 * jc : 128 * (jc + 1)],
                        wm[k],
                        start=(k == 0),
                        stop=(k == HK - 1),
                    )
                qcol = small.tile([128, 1], f32, tag="qwm", name=f"qwm_{b}_{jc}")
                nc.vector.tensor_copy(qcol, qws)
                qwm_sb.append(qcol)
            # pwm[i] = sum_h p[i,h] Wm[h]  (row), broadcast across partitions
            pws2 = pstiny.tile([1, T], f32, tag="pstiny", name=f"pwm_{b}")
            for k in range(HK):
                nc.tensor.matmul(
                    pws2, wm[k], pT[:, k, :], start=(k == 0), stop=(k == HK - 1)
                )
            pwm_row = small.tile([1, T], f16, tag="pwm_row", name=f"pwmr_{b}")
            nc.vector.tensor_copy(pwm_row, pws2)
            pwm_bc = ps256.tile([128, T], f32, tag="ps256", name=f"pwmb_{b}")
            nc.tensor.matmul(pwm_bc, ones_row, pwm_row, start=True, stop=True)
            e_sm = epool.tile([128, TC, T], f16, tag="e", name=f"esm_{b}")
            for jc in range(TC):
                # tanh(qwm[j] - pwm[i]) = Tanh(-1 * pwm_bc + qwm_col)
                tmp = epool.tile([128, T], f32, tag="tmp", name=f"tsm_{b}_{jc}")
                nc.scalar.activation(
                    tmp, pwm_bc, AF.Tanh, bias=qwm_sb[jc], scale=-1.0
                )
                nc.vector.tensor_scalar_mul(tmp, tmp, vm_c[jc])
                nc.scalar.activation(e_sm[:, jc, :], tmp, AF.Exp)
            softmax_norm_and_out(3, e_sm, qn, ones_col)

    nc.compile()
    return nc


def _get_program():
    if "nc" not in _CACHE:
        _CACHE["nc"] = _build_program()
    return _CACHE["nc"]


def kernel(**inputs):
    global last_exec_time_ns, last_trace_dir
    from concourse.bass_utils import run_bass_kernel_spmd

    nc = _get_program()

    q = np.ascontiguousarray(np.asarray(inputs["q"], dtype=np.float32))
    p = np.ascontiguousarray(np.asarray(inputs["p"], dtype=np.float32))
    weights = {
        k: np.ascontiguousarray(np.asarray(inputs[k], dtype=np.float32))
        for k in ["Wc1", "Wc2", "vc", "Wb", "Wd", "vd", "Wm", "vm"]
    }

    in_maps = []
    for i in range(NCORES):
        m = {"q": q[i * BPC : (i + 1) * BPC], "p": p[i * BPC : (i + 1) * BPC]}
        m.update(weights)
        in_maps.append(m)

    trace = bool(int(os.environ.get("BASS_KERNEL_TRACE", "0")))
    kw = {}
    if trace:
        kw.update(trace=True)
        tmpdir = os.environ.get("BASS_KERNEL_TRACE_DIR")
        if tmpdir:
            os.makedirs(tmpdir, exist_ok=True)
            kw.update(tmpdir=tmpdir)
    res = run_bass_kernel_spmd(nc, in_maps, core_ids=list(range(NCORES)), **kw)
    last_exec_time_ns = getattr(res, "exec_time_ns", None)
    results = res.results

    outs = [np.empty((B, T, H), dtype=np.float32) for _ in range(4)]
    for i in range(NCORES):
        o = results[i]["out"]
        for a in range(4):
            outs[a][i * BPC : (i + 1) * BPC] = o[a]
    return tuple(outs)
